# revision 2
# baseline (speedup 1.0000x reference)
"""Grouped-Query Attention on 8 Trainium2 NeuronCores — v2.

Sharding: core c handles (batch b = c//4, query-head group g = c%4).
Host sums the 4 group partials per batch and adds b_O + tile(b_V)@W_O.

v2 schedule: the Activation engine (exp of 4 heads x 2048^2 scores =
16.8M elements/core at ~0.83ns/col) is the bottleneck (~125us floor).
The kernel is organized to keep ACT 100% fed from ~10us in:
  - scores land in fp16 PSUM tiles [128, 2048] (2 banks x 2 bufs) so
    each exp instruction covers 2048 columns (64 activations total).
  - per query super-block sb (512 queries), a software-pipelined t-loop
    issues: scores(t+1) one step ahead, per-step filler work (x
    transpose/K/V/Q projections in sb0; prev-sb output projection +
    next-sb Q projection in sb1-3), then ctx(t) accumulation.
  - ctx accumulates 4 heads into 2 one-bank PSUM tiles (a head PAIR per
    [128, 512] fp32 bank; no ones-column). Softmax denominators come
    from 1-column matmul chains against a ones vector at sb end.
  - K projection uses column-duplicated weights so one chain writes
    both KT partition halves (no SBUF->SBUF dup DMA).
Engine budget: PE ~101us, ACT ~125us, DVE ~37us, Pool ~34us.
"""

import numpy as np

S = 2048
DM = 1024
G = 4
H = 4  # heads per group
DK = 64
GQ = 256  # query width per group
B = 2
NK = DM // 128  # 8 contraction chunks
NT = S // 128  # 16 token chunks
SBLK = 512
NSB = S // SBLK  # 4 query super-blocks

_CACHED = {}


def _split_sync_waits(nc, drain_max=1, other_max=1):
    """This walrus build has a single sync-wait slot on CTRL-class
    instructions (Drain/NoOp); Tile's exit drain collects 3+. Move the
    excess onto preceding single-wait NOPs on the same engine."""
    import concourse.mybir as mybir
    import bass_rust

    n_split = 0
    for f in nc.m.functions:
        for bb in f.blocks:
            out = []
            changed = False
            for inst in bb.instructions:
                si = getattr(inst, "sync_info", None)
                limit = drain_max if type(inst).__name__ in ("InstDrain", "InstNoOp") else other_max
                if si is not None and len(si.on_wait) > limit:
                    waits = list(si.on_wait)
                    keep = waits[-limit:] if limit else []
                    head = waits[: len(waits) - limit]
                    for w in head:
                        out.append(
                            mybir.InstNoOp(
                                name=f"{inst.name}-wsp{n_split}",
                                engine=inst.engine,
                                sync_info=mybir.SyncInfo(on_wait=[w], on_update=[]),
                                bass_nofuse=True,
                            )
                        )
                        n_split += 1
                    inst.sync_info = bass_rust.SyncInfo(on_wait=keep, on_update=si.on_update)
                    changed = True
                out.append(inst)
            if changed:
                bb.instructions = out
    return n_split


def _build_nc(iters=1):
    import concourse.bass as bass
    import concourse.mybir as mybir
    import concourse.tile as tile
    from concourse import masks

    F32 = mybir.dt.float32
    F16 = mybir.dt.float16
    BF = mybir.dt.bfloat16
    EXP = mybir.ActivationFunctionType.Exp

    nc = bass.Bass("TRN2", target_bir_lowering=False, debug=False, num_devices=8)
    x = nc.dram_tensor("x", [S, DM], F32, kind="ExternalInput")
    wq = nc.dram_tensor("wq", [DM, GQ], F32, kind="ExternalInput")
    wk = nc.dram_tensor("wk", [DM, DK], F32, kind="ExternalInput")
    wv = nc.dram_tensor("wv", [DM, DK], F32, kind="ExternalInput")
    wo = nc.dram_tensor("wo", [GQ, DM], F32, kind="ExternalInput")
    bq = nc.dram_tensor("bq", [GQ], F32, kind="ExternalInput")
    bk = nc.dram_tensor("bk", [DK], F32, kind="ExternalInput")
    out = nc.dram_tensor("out", [S, DM], F32, kind="ExternalOutput")

    with tile.TileContext(nc) as tc:
        with (
            tc.tile_pool(name="const", bufs=1) as cpool,
            tc.tile_pool(name="wstg", bufs=2) as wstg,
            tc.tile_pool(name="wts", bufs=1) as wts,
            tc.tile_pool(name="xin", bufs=2) as xin,
            tc.tile_pool(name="acts", bufs=1) as acts,
            tc.tile_pool(name="outp", bufs=2) as outp,
            tc.tile_pool(name="ps_big", bufs=2, space="PSUM") as ps_big,
            tc.tile_pool(name="ps_ctx", bufs=2, space="PSUM") as ps_ctx,
            tc.tile_pool(name="ps_proj", bufs=2, space="PSUM") as ps_proj,
        ):
            def _pipeline():
                # ---- constants ----
                ident_b = cpool.tile([128, 128], BF)
                masks.make_identity(nc, ident_b[:])
                ones_b = cpool.tile([128, 1], BF)
                nc.vector.memset(ones_b[:], 1.0)
                bq_t = cpool.tile([128, 2], F32)
                for m in range(2):
                    nc.sync.dma_start(bq_t[:, m : m + 1], bq[m * 128 : (m + 1) * 128])
                bk_t = cpool.tile([128, 1], F32)
                nc.sync.dma_start(bk_t[:64, :], bk[:])
                nc.sync.dma_start(bk_t[64:128, :], bk[:])

                # ---- persistent SBUF tensors ----
                xT = acts.tile([128, NK * S], BF)  # [:, k*2048 + tok]
                QT = acts.tile([128, 2 * S], BF)  # [:, m*2048 + sb*512 + q]
                KT = acts.tile([128, S], BF)  # rows 64-127 duplicate 0-63
                Vt = acts.tile([128, NT * DK], BF)  # [:, t*64 + d]
                PT = acts.tile([128, NT * H * SBLK], BF)  # [:, (t*4+h)*512 + q]
                ctxs = acts.tile([128, 4 * GQ], BF)  # [:, q4*256 + h*64 + d]
                ctxTs = acts.tile([128, 2 * SBLK], BF)  # [:, cj*512 + q4*128 + s]

                # ---- weights: stage fp32, cast to bf16 on Pool ----
                wkd_bf = wts.tile([128, NK * 128], BF)  # [wk|wk] per k chunk
                wq_bf = wts.tile([128, NK * GQ], BF)
                wv_bf = wts.tile([128, NK * DK], BF)
                wo_bf = wts.tile([128, 2 * DM], BF)

                def stage_wk():
                    stgk = wstg.tile([128, 1024], F32, tag="wstg", name="stgk")
                    for u in range(2):
                        nc.sync.dma_start(
                            stgk[:].rearrange("p (k u c) -> p k u c", u=2, c=DK)[
                                :, :, u, :
                            ],
                            wk[:].rearrange("(k p) c -> p k c", p=128),
                        )
                    nc.gpsimd.tensor_copy(wkd_bf[:], stgk[:])

                def stage_wq():
                    for j in range(2):
                        stgq = wstg.tile([128, 1024], F32, tag="wstg", name="stgq")
                        nc.sync.dma_start(
                            stgq[:].rearrange("p (k q) -> p k q", q=GQ),
                            wq[4 * j * 128 : 4 * (j + 1) * 128, :].rearrange(
                                "(k p) q -> p k q", p=128
                            ),
                        )
                        nc.gpsimd.tensor_copy(
                            wq_bf[:, j * 1024 : (j + 1) * 1024], stgq[:]
                        )

                # ---- x: load + cast + transpose, half-superblock at a time ----
                def x_load(hs):
                    xf = xin.tile([128, 2048], F32, tag="xf", name="xf")
                    nc.sync.dma_start(
                        xf[:].rearrange("p (i d) -> p i d", d=DM),
                        x[hs * 256 : (hs + 1) * 256, :].rearrange(
                            "(i p) d -> p i d", p=128
                        ),
                    )
                    return xf

                def x_tr(hs, xf, cast_eng=None):
                    xbf = xin.tile([128, 2048], BF, tag="xbf", name="xbf")
                    (cast_eng or nc.gpsimd).tensor_copy(xbf[:], xf[:])
                    ps = ps_big.tile([128, 2048], BF, tag="sc", name="pstr")
                    for i in range(2):
                        for k in range(NK):
                            nc.tensor.transpose(
                                ps[:, (i * NK + k) * 128 : (i * NK + k + 1) * 128],
                                xbf[:, i * DM + k * 128 : i * DM + (k + 1) * 128],
                                ident_b[:],
                            )
                    nc.vector.tensor_copy(
                        xT[:].rearrange("p (k j c) -> p k j c", k=NK, j=NT)[
                            :, :, 2 * hs : 2 * hs + 2, :
                        ],
                        ps[:].rearrange("p (i k c) -> p k i c", i=2, k=NK),
                    )

                def k_proj(hs):
                    pk = ps_proj.tile([128, 256], F32, tag="p", name="pk")
                    for k in range(NK):
                        nc.tensor.matmul(
                            pk[:],
                            wkd_bf[:, k * 128 : (k + 1) * 128],
                            xT[:, k * S + hs * 256 : k * S + (hs + 1) * 256],
                            start=(k == 0),
                            stop=(k == NK - 1),
                        )
                    nc.vector.tensor_scalar_add(
                        KT[:, hs * 256 : (hs + 1) * 256], pk[:], bk_t[:]
                    )

                def q_proj(sb, m):
                    pq = ps_proj.tile([128, SBLK], F32, tag="p", name="pq")
                    for k in range(NK):
                        nc.tensor.matmul(
                            pq[:],
                            wq_bf[:, k * GQ + m * 128 : k * GQ + (m + 1) * 128],
                            xT[:, k * S + sb * SBLK : k * S + (sb + 1) * SBLK],
                            start=(k == 0),
                            stop=(k == NK - 1),
                        )
                    nc.vector.tensor_scalar_add(
                        QT[:, m * S + sb * SBLK : m * S + (sb + 1) * SBLK],
                        pq[:],
                        bq_t[:, m : m + 1],
                    )

                def v_proj(t):
                    pv = ps_proj.tile([128, DK], F32, tag="p", name="pv")
                    for k in range(NK):
                        nc.tensor.matmul(
                            pv[:],
                            xT[:, k * S + t * 128 : k * S + (t + 1) * 128],
                            wv_bf[:, k * DK : (k + 1) * DK],
                            start=(k == 0),
                            stop=(k == NK - 1),
                        )
                    nc.vector.tensor_copy(Vt[:, t * DK : (t + 1) * DK], pv[:])

                def stage_wv():
                    stgv = wstg.tile([128, 512], F32, tag="wstg", name="stgv")
                    nc.sync.dma_start(
                        stgv[:, :512].rearrange("p (k c) -> p k c", c=DK),
                        wv[:].rearrange("(k p) c -> p k c", p=128),
                    )
                    nc.gpsimd.tensor_copy(wv_bf[:], stgv[:, :512])

                def stage_wo():
                    for cj in range(2):
                        stgo = wstg.tile([128, 1024], F32, tag="wstg", name="stgo")
                        nc.sync.dma_start(stgo[:], wo[cj * 128 : (cj + 1) * 128, :])
                        nc.gpsimd.tensor_copy(
                            wo_bf[:, cj * DM : (cj + 1) * DM], stgo[:]
                        )

                # ---- attention building blocks ----
                def sc_issue(sb, t):
                    # fp32 PSUM (matmul can only write fp32): one 2-bank tile
                    # and one exp instruction per head-pair m.
                    for m in range(2):
                        scp = ps_big.tile([128, 2 * SBLK], F32, tag="sc", name="scp")
                        for hl in range(2):
                            nc.tensor.matmul(
                                scp[:, hl * SBLK : (hl + 1) * SBLK],
                                KT[hl * 64 : (hl + 1) * 64, t * 128 : (t + 1) * 128],
                                QT[hl * 64 : (hl + 1) * 64,
                                   m * S + sb * SBLK : m * S + (sb + 1) * SBLK],
                            )
                        nc.scalar.activation(
                            PT[:, (t * H + 2 * m) * SBLK : (t * H + 2 * m + 2) * SBLK],
                            scp[:],
                            EXP,
                            scale=0.125,
                        )

                # ctx/denominator accumulate over t in quarter-length COMPLETE
                # psum chains (hw allows only one open accumulation chain per
                # PSUM bank at a time), summed across quarters into SBUF fp32
                # accumulators by DVE.
                cacc = acts.tile([128, 1024], F32)  # pair i at cols i*512
                dacc = acts.tile([128, 16], F32)  # col h*4+q4

                MUL = mybir.AluOpType.mult
                ADD = mybir.AluOpType.add

                def ctx_quarter(sb, tq):
                    t0 = 4 * tq
                    pairs = [
                        ps_ctx.tile([128, 512], F32, tag="c", name=f"pr{tq}{i}")
                        for i in range(2)
                    ]
                    dn = ps_proj.tile([128, 16], F32, tag="p", name="dn")
                    for h in range(H):
                        for q4 in range(4):
                            for t in range(t0, t0 + 4):
                                nc.tensor.matmul(
                                    pairs[h // 2][:, (h % 2) * 256 + q4 * DK :
                                                  (h % 2) * 256 + (q4 + 1) * DK],
                                    PT[:, (t * H + h) * SBLK + q4 * 128 :
                                       (t * H + h) * SBLK + (q4 + 1) * 128],
                                    Vt[:, t * DK : (t + 1) * DK],
                                    start=(t == t0),
                                    stop=(t == t0 + 3),
                                )
                            for t in range(t0, t0 + 4):
                                nc.tensor.matmul(
                                    dn[:, h * 4 + q4 : h * 4 + q4 + 1],
                                    PT[:, (t * H + h) * SBLK + q4 * 128 :
                                       (t * H + h) * SBLK + (q4 + 1) * 128],
                                    ones_b[:],
                                    start=(t == t0),
                                    stop=(t == t0 + 3),
                                )
                    for i in range(2):
                        if tq == 0:
                            nc.vector.tensor_copy(
                                cacc[:, i * 512 : (i + 1) * 512], pairs[i][:]
                            )
                        else:
                            nc.vector.scalar_tensor_tensor(
                                cacc[:, i * 512 : (i + 1) * 512],
                                pairs[i][:],
                                1.0,
                                cacc[:, i * 512 : (i + 1) * 512],
                                MUL,
                                ADD,
                            )
                    if tq == 0:
                        nc.vector.tensor_copy(dacc[:], dn[:])
                    else:
                        nc.vector.scalar_tensor_tensor(
                            dacc[:], dn[:], 1.0, dacc[:], MUL, ADD
                        )

                def ctx_finish():
                    rc_t = xin.tile([128, 16], F32, tag="rc", name="rc")
                    nc.vector.reciprocal(rc_t[:], dacc[:])
                    for h in range(H):
                        for q4 in range(4):
                            nc.vector.tensor_scalar_mul(
                                ctxs[:, q4 * GQ + h * DK : q4 * GQ + (h + 1) * DK],
                                cacc[:, (h // 2) * 512 + (h % 2) * 256 + q4 * DK :
                                     (h // 2) * 512 + (h % 2) * 256 + (q4 + 1) * DK],
                                rc_t[:, h * 4 + q4 : h * 4 + q4 + 1],
                            )

                def ctxT_issue():
                    for cj in range(2):
                        pct = ps_proj.tile([128, SBLK], BF, tag="p", name="pct")
                        for q4 in range(4):
                            nc.tensor.transpose(
                                pct[:, q4 * 128 : (q4 + 1) * 128],
                                ctxs[:, q4 * GQ + cj * 128 : q4 * GQ + (cj + 1) * 128],
                                ident_b[:],
                            )
                        nc.vector.tensor_copy(
                            ctxTs[:, cj * SBLK : (cj + 1) * SBLK], pct[:]
                        )

                def out_issue(sb, half):
                    ot = outp.tile([128, 2 * DM], F32, tag="ot", name="ot")
                    for ci in range(2):
                        q4 = half * 2 + ci
                        for nb in range(2):
                            po = ps_proj.tile([128, SBLK], F32, tag="p", name="po")
                            for cj in range(2):
                                nc.tensor.matmul(
                                    po[:],
                                    ctxTs[:, cj * SBLK + q4 * 128 :
                                          cj * SBLK + (q4 + 1) * 128],
                                    wo_bf[:, cj * DM + nb * SBLK :
                                          cj * DM + (nb + 1) * SBLK],
                                    start=(cj == 0),
                                    stop=(cj == 1),
                                )
                            nc.vector.tensor_copy(
                                ot[:, ci * DM + nb * SBLK : ci * DM + (nb + 1) * SBLK],
                                po[:],
                            )
                    row = sb * SBLK + half * 256
                    nc.sync.dma_start(
                        out[row : row + 256, :].rearrange("(c p) d -> p c d", p=128),
                        ot[:].rearrange("p (c d) -> p c d", d=DM),
                    )

                # ---- phase 0: head ----
                # SP DMA ring order: x blocks 0-1 first (critical path), then
                # wk/wq/wv, remaining x blocks, wo. The first two x casts run
                # on DVE so Pool's weight casts don't serialize the head.
                xfs = {}
                xfs[0] = x_load(0)
                xfs[1] = x_load(1)
                stage_wk()
                stage_wq()
                stage_wv()
                x_tr(0, xfs[0], cast_eng=nc.vector)
                x_tr(1, xfs[1], cast_eng=nc.vector)
                k_proj(0)
                k_proj(1)
                q_proj(0, 0)
                q_proj(0, 1)
                v_proj(0)
                v_proj(1)
                # Remaining x DMAs + wo staging go on the SP ring now; the
                # per-step fillers below consume them as they land.
                for hs in range(2, 8):
                    xfs[hs] = x_load(hs)
                stage_wo()

                # ---- per-sb filler schedules ----
                def xtr_k(hs):
                    x_tr(hs, xfs[hs])
                    k_proj(hs)

                def sb0_fillers():
                    f = {}
                    for j, hs in ((0, 2), (1, 3), (4, 4), (5, 5), (8, 6), (9, 7)):
                        f[j] = [lambda hs=hs: xtr_k(hs)]
                    f[2] = [lambda: v_proj(2), lambda: v_proj(3)]
                    f[3] = [lambda: v_proj(4), lambda: v_proj(5)]
                    f[6] = [lambda: v_proj(6), lambda: v_proj(7)]
                    f[7] = [lambda: v_proj(8), lambda: v_proj(9)]
                    f[10] = [lambda: v_proj(10), lambda: v_proj(11)]
                    f[11] = [lambda: v_proj(12), lambda: v_proj(13)]
                    f[12] = [lambda: q_proj(1, 0), lambda: v_proj(14)]
                    f[13] = [lambda: q_proj(1, 1), lambda: v_proj(15)]
                    return f

                def sb_fillers(sb):
                    f = {
                        0: [ctx_finish],
                        1: [ctxT_issue],
                        2: [lambda: out_issue(sb - 1, 0)],
                        4: [lambda: out_issue(sb - 1, 1)],
                    }
                    if sb < NSB - 1:
                        f[5] = [lambda: q_proj(sb + 1, 0)]
                        f[6] = [lambda: q_proj(sb + 1, 1)]
                    return f

                def sb_loop(sb, fillers):
                    sc_issue(sb, 0)
                    for t in range(NT):
                        if t < NT - 1:
                            sc_issue(sb, t + 1)
                        for fn in fillers.get(t, []):
                            fn()
                        if t % 4 == 3:
                            ctx_quarter(sb, t // 4)

                sb_loop(0, sb0_fillers())
                for sb in range(1, NSB):
                    sb_loop(sb, sb_fillers(sb))
                # ---- tail ----
                ctx_finish()
                ctxT_issue()
                out_issue(NSB - 1, 0)
                out_issue(NSB - 1, 1)

            if iters == 1:
                _pipeline()
            else:
                with tc.For_i(0, iters):
                    _pipeline()

    _split_sync_waits(nc)
    return nc


def kernel(x, W_Q, b_Q, W_K, b_K, W_V, b_V, W_O, b_O):
    from concourse.bass_utils import run_bass_kernel_spmd

    x = np.asarray(x, np.float32)
    W_Q, b_Q = np.asarray(W_Q, np.float32), np.asarray(b_Q, np.float32)
    W_K, b_K = np.asarray(W_K, np.float32), np.asarray(b_K, np.float32)
    W_V, b_V = np.asarray(W_V, np.float32), np.asarray(b_V, np.float32)
    W_O, b_O = np.asarray(W_O, np.float32), np.asarray(b_O, np.float32)

    if "nc" not in _CACHED:
        _CACHED["nc"] = _build_nc()
    nc = _CACHED["nc"]

    in_maps = []
    for c in range(8):
        b, g = divmod(c, 4)
        in_maps.append(
            {
                "x": np.ascontiguousarray(x[b]),
                "wq": np.ascontiguousarray(W_Q[:, g * GQ : (g + 1) * GQ]),
                "wk": np.ascontiguousarray(W_K[g]),
                "wv": np.ascontiguousarray(W_V[g]),
                "wo": np.ascontiguousarray(W_O[g * GQ : (g + 1) * GQ, :]),
                "bq": np.ascontiguousarray(b_Q[g * GQ : (g + 1) * GQ]),
                "bk": np.ascontiguousarray(b_K[g]),
            }
        )
    res = run_bass_kernel_spmd(nc, in_maps, list(range(8)))

    out = np.zeros((B, S, DM), np.float32)
    for c in range(8):
        b, g = divmod(c, 4)
        out[b] += res.results[c]["out"]
    # host-side bias terms: b_O, plus b_V's exact contribution
    # (softmax rows sum to 1 -> ctx bias = tile(b_V[g]) per head)
    bv_full = np.concatenate([np.tile(b_V[g], H) for g in range(G)])  # [1024]
    out += (b_O + bv_full @ W_O)[None, None, :]
    return out


# revision 3
# speedup vs baseline: 1.3530x; 1.3530x over previous
"""Grouped-Query Attention on 8 Trainium2 NeuronCores — v2.

Sharding: core c handles (batch b = c//4, query-head group g = c%4).
Host sums the 4 group partials per batch and adds b_O + tile(b_V)@W_O.

v2 schedule: the Activation engine (exp of 4 heads x 2048^2 scores =
16.8M elements/core at ~0.83ns/col) is the bottleneck (~125us floor).
The kernel is organized to keep ACT 100% fed from ~10us in:
  - scores land in fp16 PSUM tiles [128, 2048] (2 banks x 2 bufs) so
    each exp instruction covers 2048 columns (64 activations total).
  - per query super-block sb (512 queries), a software-pipelined t-loop
    issues: scores(t+1) one step ahead, per-step filler work (x
    transpose/K/V/Q projections in sb0; prev-sb output projection +
    next-sb Q projection in sb1-3), then ctx(t) accumulation.
  - ctx accumulates 4 heads into 2 one-bank PSUM tiles (a head PAIR per
    [128, 512] fp32 bank; no ones-column). Softmax denominators come
    from 1-column matmul chains against a ones vector at sb end.
  - K projection uses column-duplicated weights so one chain writes
    both KT partition halves (no SBUF->SBUF dup DMA).
Engine budget: PE ~101us, ACT ~125us, DVE ~37us, Pool ~34us.
"""

import numpy as np

S = 2048
DM = 1024
G = 4
H = 4  # heads per group
DK = 64
GQ = 256  # query width per group
B = 2
NK = DM // 128  # 8 contraction chunks
NT = S // 128  # 16 token chunks
SBLK = 512
NSB = S // SBLK  # 4 query super-blocks

_CACHED = {}


def _split_sync_waits(nc, drain_max=1, other_max=1):
    """This walrus build has a single sync-wait slot on CTRL-class
    instructions (Drain/NoOp); Tile's exit drain collects 3+. Move the
    excess onto preceding single-wait NOPs on the same engine."""
    import concourse.mybir as mybir
    import bass_rust

    n_split = 0
    for f in nc.m.functions:
        for bb in f.blocks:
            out = []
            changed = False
            for inst in bb.instructions:
                si = getattr(inst, "sync_info", None)
                limit = drain_max if type(inst).__name__ in ("InstDrain", "InstNoOp") else other_max
                if si is not None and len(si.on_wait) > limit:
                    waits = list(si.on_wait)
                    keep = waits[-limit:] if limit else []
                    head = waits[: len(waits) - limit]
                    for w in head:
                        out.append(
                            mybir.InstNoOp(
                                name=f"{inst.name}-wsp{n_split}",
                                engine=inst.engine,
                                sync_info=mybir.SyncInfo(on_wait=[w], on_update=[]),
                                bass_nofuse=True,
                            )
                        )
                        n_split += 1
                    inst.sync_info = bass_rust.SyncInfo(on_wait=keep, on_update=si.on_update)
                    changed = True
                out.append(inst)
            if changed:
                bb.instructions = out
    return n_split


def _build_nc(iters=1):
    import concourse.bass as bass
    import concourse.mybir as mybir
    import concourse.tile as tile
    from concourse import masks

    F32 = mybir.dt.float32
    F16 = mybir.dt.float16
    BF = mybir.dt.bfloat16
    EXP = mybir.ActivationFunctionType.Exp

    nc = bass.Bass("TRN2", target_bir_lowering=False, debug=False, num_devices=8)
    x = nc.dram_tensor("x", [S, DM], F32, kind="ExternalInput")
    wq = nc.dram_tensor("wq", [DM, GQ], F32, kind="ExternalInput")
    wk = nc.dram_tensor("wk", [DM, DK], F32, kind="ExternalInput")
    wv = nc.dram_tensor("wv", [DM, DK], F32, kind="ExternalInput")
    wo = nc.dram_tensor("wo", [GQ, DM], F32, kind="ExternalInput")
    bq = nc.dram_tensor("bq", [GQ], F32, kind="ExternalInput")
    bk = nc.dram_tensor("bk", [DK], F32, kind="ExternalInput")
    out = nc.dram_tensor("out", [S, DM], F32, kind="ExternalOutput")

    with tile.TileContext(nc) as tc:
        with (
            tc.tile_pool(name="const", bufs=1) as cpool,
            tc.tile_pool(name="wstg", bufs=2) as wstg,
            tc.tile_pool(name="wts", bufs=1) as wts,
            tc.tile_pool(name="xin", bufs=2) as xin,
            tc.tile_pool(name="acts", bufs=1) as acts,
            tc.tile_pool(name="outp", bufs=2) as outp,
            tc.tile_pool(name="ps_big", bufs=2, space="PSUM") as ps_big,
            tc.tile_pool(name="ps_ctx", bufs=2, space="PSUM") as ps_ctx,
            tc.tile_pool(name="ps_proj", bufs=2, space="PSUM") as ps_proj,
        ):
            def _pipeline():
                # ---- constants ----
                ident_b = cpool.tile([128, 128], BF)
                masks.make_identity(nc, ident_b[:])
                ones_b = cpool.tile([128, 1], BF)
                nc.vector.memset(ones_b[:], 1.0)
                bq_t = cpool.tile([128, 2], F32)
                for m in range(2):
                    nc.sync.dma_start(bq_t[:, m : m + 1], bq[m * 128 : (m + 1) * 128])
                bk_t = cpool.tile([128, 1], F32)
                nc.sync.dma_start(bk_t[:64, :], bk[:])
                nc.sync.dma_start(bk_t[64:128, :], bk[:])

                # ---- persistent SBUF tensors ----
                xT = acts.tile([128, NK * S], BF)  # [:, k*2048 + tok]
                QT = acts.tile([128, 2 * S], BF)  # [:, m*2048 + sb*512 + q]
                KT = acts.tile([128, S], BF)  # rows 64-127 duplicate 0-63
                Vt = acts.tile([128, NT * 65], BF)  # [:, t*65 + d]; col 64 = ones
                PT = acts.tile([128, NT * H * SBLK], BF)  # [:, (t*4+h)*512 + q]
                ctxs = acts.tile([128, 4 * GQ], BF)  # [:, q4*256 + h*64 + d]
                ctxTs = acts.tile([128, 2 * SBLK], BF)  # [:, cj*512 + q4*128 + s]

                # ---- weights: stage fp32, cast to bf16 on Pool ----
                wkd_bf = wts.tile([128, NK * 128], BF)  # [wk|wk] per k chunk
                wq_bf = wts.tile([128, NK * GQ], BF)
                wv_bf = wts.tile([128, NK * DK], BF)
                wo_bf = wts.tile([128, 2 * DM], BF)

                def stage_wk():
                    stgk = wstg.tile([128, 1024], F32, tag="wstg", name="stgk")
                    for u in range(2):
                        nc.sync.dma_start(
                            stgk[:].rearrange("p (k u c) -> p k u c", u=2, c=DK)[
                                :, :, u, :
                            ],
                            wk[:].rearrange("(k p) c -> p k c", p=128),
                        )
                    nc.gpsimd.tensor_copy(wkd_bf[:], stgk[:])

                def stage_wq():
                    for j in range(2):
                        stgq = wstg.tile([128, 1024], F32, tag="wstg", name="stgq")
                        nc.sync.dma_start(
                            stgq[:].rearrange("p (k q) -> p k q", q=GQ),
                            wq[4 * j * 128 : 4 * (j + 1) * 128, :].rearrange(
                                "(k p) q -> p k q", p=128
                            ),
                        )
                        nc.gpsimd.tensor_copy(
                            wq_bf[:, j * 1024 : (j + 1) * 1024], stgq[:]
                        )

                # ---- x: load + cast + transpose, half-superblock at a time ----
                def x_load(hs):
                    xf = xin.tile([128, 2048], F32, tag="xf", name="xf")
                    nc.sync.dma_start(
                        xf[:].rearrange("p (i d) -> p i d", d=DM),
                        x[hs * 256 : (hs + 1) * 256, :].rearrange(
                            "(i p) d -> p i d", p=128
                        ),
                    )
                    return xf

                def x_tr(hs, xf, cast_eng=None):
                    xbf = xin.tile([128, 2048], BF, tag="xbf", name="xbf")
                    (cast_eng or nc.gpsimd).tensor_copy(xbf[:], xf[:])
                    ps = ps_big.tile([128, 2048], BF, tag="sc", name="pstr")
                    for i in range(2):
                        for k in range(NK):
                            nc.tensor.transpose(
                                ps[:, (i * NK + k) * 128 : (i * NK + k + 1) * 128],
                                xbf[:, i * DM + k * 128 : i * DM + (k + 1) * 128],
                                ident_b[:],
                            )
                    nc.vector.tensor_copy(
                        xT[:].rearrange("p (k j c) -> p k j c", k=NK, j=NT)[
                            :, :, 2 * hs : 2 * hs + 2, :
                        ],
                        ps[:].rearrange("p (i k c) -> p k i c", i=2, k=NK),
                    )

                def k_proj(hs):
                    pk = ps_proj.tile([128, 256], F32, tag="p", name="pk")
                    for k in range(NK):
                        nc.tensor.matmul(
                            pk[:],
                            wkd_bf[:, k * 128 : (k + 1) * 128],
                            xT[:, k * S + hs * 256 : k * S + (hs + 1) * 256],
                            start=(k == 0),
                            stop=(k == NK - 1),
                        )
                    nc.vector.tensor_scalar_add(
                        KT[:, hs * 256 : (hs + 1) * 256], pk[:], bk_t[:]
                    )

                def q_proj(sb, m):
                    pq = ps_proj.tile([128, SBLK], F32, tag="p", name="pq")
                    for k in range(NK):
                        nc.tensor.matmul(
                            pq[:],
                            wq_bf[:, k * GQ + m * 128 : k * GQ + (m + 1) * 128],
                            xT[:, k * S + sb * SBLK : k * S + (sb + 1) * SBLK],
                            start=(k == 0),
                            stop=(k == NK - 1),
                        )
                    nc.vector.tensor_scalar_add(
                        QT[:, m * S + sb * SBLK : m * S + (sb + 1) * SBLK],
                        pq[:],
                        bq_t[:, m : m + 1],
                    )

                def v_proj(t):
                    pv = ps_proj.tile([128, DK], F32, tag="p", name="pv")
                    for k in range(NK):
                        nc.tensor.matmul(
                            pv[:],
                            xT[:, k * S + t * 128 : k * S + (t + 1) * 128],
                            wv_bf[:, k * DK : (k + 1) * DK],
                            start=(k == 0),
                            stop=(k == NK - 1),
                        )
                    nc.vector.tensor_copy(Vt[:, t * 65 : t * 65 + DK], pv[:])
                    nc.vector.memset(Vt[:, t * 65 + DK : t * 65 + 65], 1.0)

                def stage_wv():
                    stgv = wstg.tile([128, 512], F32, tag="wstg", name="stgv")
                    nc.sync.dma_start(
                        stgv[:, :512].rearrange("p (k c) -> p k c", c=DK),
                        wv[:].rearrange("(k p) c -> p k c", p=128),
                    )
                    nc.gpsimd.tensor_copy(wv_bf[:], stgv[:, :512])

                def stage_wo():
                    for cj in range(2):
                        stgo = wstg.tile([128, 1024], F32, tag="wstg", name="stgo")
                        nc.sync.dma_start(stgo[:], wo[cj * 128 : (cj + 1) * 128, :])
                        nc.gpsimd.tensor_copy(
                            wo_bf[:, cj * DM : (cj + 1) * DM], stgo[:]
                        )

                # ---- attention building blocks ----
                def sc_issue(sb, t):
                    # fp32 PSUM (matmul can only write fp32): one 2-bank tile
                    # and one exp instruction per head-pair m.
                    for m in range(2):
                        scp = ps_big.tile([128, 2 * SBLK], F32, tag="sc", name="scp")
                        for hl in range(2):
                            nc.tensor.matmul(
                                scp[:, hl * SBLK : (hl + 1) * SBLK],
                                KT[hl * 64 : (hl + 1) * 64, t * 128 : (t + 1) * 128],
                                QT[hl * 64 : (hl + 1) * 64,
                                   m * S + sb * SBLK : m * S + (sb + 1) * SBLK],
                            )
                        nc.scalar.activation(
                            PT[:, (t * H + 2 * m) * SBLK : (t * H + 2 * m + 2) * SBLK],
                            scp[:],
                            EXP,
                            scale=0.125,
                        )

                # ctx/denominator accumulate over t in quarter-length COMPLETE
                # psum chains (hw allows only one open accumulation chain per
                # PSUM bank at a time), summed across quarters into SBUF fp32
                # accumulators by DVE.
                # cacc col layout: h*260 + q4*65 + d; d==64 is the softmax
                # denominator (65th ones-column of Vt rides each ctx matmul).
                cacc = acts.tile([128, H * 260], F32)

                MUL = mybir.AluOpType.mult
                ADD = mybir.AluOpType.add

                def ctx_quarter(sb, tq):
                    t0 = 4 * tq
                    tiles = []
                    for h in range(H):
                        ct = ps_ctx.tile([128, 260], F32, tag="c", name=f"ct{tq}{h}")
                        for q4 in range(4):
                            for t in range(t0, t0 + 4):
                                nc.tensor.matmul(
                                    ct[:, q4 * 65 : (q4 + 1) * 65],
                                    PT[:, (t * H + h) * SBLK + q4 * 128 :
                                       (t * H + h) * SBLK + (q4 + 1) * 128],
                                    Vt[:, t * 65 : (t + 1) * 65],
                                    start=(t == t0),
                                    stop=(t == t0 + 3),
                                )
                        if tq == 0:
                            nc.vector.tensor_copy(
                                cacc[:, h * 260 : (h + 1) * 260], ct[:]
                            )
                        else:
                            nc.vector.scalar_tensor_tensor(
                                cacc[:, h * 260 : (h + 1) * 260],
                                ct[:],
                                1.0,
                                cacc[:, h * 260 : (h + 1) * 260],
                                MUL,
                                ADD,
                            )
                        tiles.append(ct)

                def ctx_finish():
                    rc_t = xin.tile([128, 16], F32, tag="rc", name="rc")
                    nc.vector.reciprocal(
                        rc_t[:],
                        cacc[:].rearrange("p (h q c) -> p h q c", h=H, c=65)[:, :, :, DK],
                    )
                    for h in range(H):
                        for q4 in range(4):
                            nc.vector.tensor_scalar_mul(
                                ctxs[:, q4 * GQ + h * DK : q4 * GQ + (h + 1) * DK],
                                cacc[:, h * 260 + q4 * 65 : h * 260 + q4 * 65 + DK],
                                rc_t[:, h * 4 + q4 : h * 4 + q4 + 1],
                            )

                def ctxT_issue():
                    for cj in range(2):
                        pct = ps_proj.tile([128, SBLK], BF, tag="p", name="pct")
                        for q4 in range(4):
                            nc.tensor.transpose(
                                pct[:, q4 * 128 : (q4 + 1) * 128],
                                ctxs[:, q4 * GQ + cj * 128 : q4 * GQ + (cj + 1) * 128],
                                ident_b[:],
                            )
                        nc.vector.tensor_copy(
                            ctxTs[:, cj * SBLK : (cj + 1) * SBLK], pct[:]
                        )

                def out_issue(sb, half):
                    ot = outp.tile([128, 2 * DM], F32, tag="ot", name="ot")
                    for ci in range(2):
                        q4 = half * 2 + ci
                        for nb in range(2):
                            po = ps_proj.tile([128, SBLK], F32, tag="p", name="po")
                            for cj in range(2):
                                nc.tensor.matmul(
                                    po[:],
                                    ctxTs[:, cj * SBLK + q4 * 128 :
                                          cj * SBLK + (q4 + 1) * 128],
                                    wo_bf[:, cj * DM + nb * SBLK :
                                          cj * DM + (nb + 1) * SBLK],
                                    start=(cj == 0),
                                    stop=(cj == 1),
                                )
                            nc.vector.tensor_copy(
                                ot[:, ci * DM + nb * SBLK : ci * DM + (nb + 1) * SBLK],
                                po[:],
                            )
                    row = sb * SBLK + half * 256
                    nc.sync.dma_start(
                        out[row : row + 256, :].rearrange("(c p) d -> p c d", p=128),
                        ot[:].rearrange("p (c d) -> p c d", d=DM),
                    )

                # ---- phase 0: head ----
                # SP DMA ring order: x blocks 0-1 first (critical path), then
                # wk/wq/wv, remaining x blocks, wo. The first two x casts run
                # on DVE so Pool's weight casts don't serialize the head.
                xfs = {}
                xfs[0] = x_load(0)
                xfs[1] = x_load(1)
                stage_wk()
                stage_wq()
                stage_wv()
                x_tr(0, xfs[0], cast_eng=nc.vector)
                x_tr(1, xfs[1], cast_eng=nc.vector)
                k_proj(0)
                k_proj(1)
                q_proj(0, 0)
                q_proj(0, 1)
                v_proj(0)
                v_proj(1)
                # Remaining x DMAs + wo staging go on the SP ring now; the
                # per-step fillers below consume them as they land.
                for hs in range(2, 8):
                    xfs[hs] = x_load(hs)
                stage_wo()

                # ---- per-sb filler schedules ----
                def xtr_k(hs):
                    x_tr(hs, xfs[hs])
                    k_proj(hs)

                def sb0_fillers():
                    f = {}
                    for j, hs in ((0, 2), (1, 3), (4, 4), (5, 5), (8, 6), (9, 7)):
                        f[j] = [lambda hs=hs: xtr_k(hs)]
                    f[2] = [lambda: v_proj(2), lambda: v_proj(3)]
                    f[3] = [lambda: v_proj(4), lambda: v_proj(5)]
                    f[6] = [lambda: v_proj(6), lambda: v_proj(7)]
                    f[7] = [lambda: v_proj(8), lambda: v_proj(9)]
                    f[10] = [lambda: v_proj(10), lambda: v_proj(11)]
                    f[11] = [lambda: v_proj(12), lambda: v_proj(13)]
                    f[12] = [lambda: q_proj(1, 0), lambda: v_proj(14)]
                    f[13] = [lambda: q_proj(1, 1), lambda: v_proj(15)]
                    return f

                def sb_fillers(sb):
                    f = {
                        0: [ctx_finish],
                        1: [ctxT_issue],
                        2: [lambda: out_issue(sb - 1, 0)],
                        4: [lambda: out_issue(sb - 1, 1)],
                    }
                    if sb < NSB - 1:
                        f[5] = [lambda: q_proj(sb + 1, 0)]
                        f[6] = [lambda: q_proj(sb + 1, 1)]
                    return f

                def sb_loop(sb, fillers):
                    sc_issue(sb, 0)
                    for t in range(NT):
                        if t < NT - 1:
                            sc_issue(sb, t + 1)
                        for fn in fillers.get(t, []):
                            fn()
                        if t % 4 == 3:
                            ctx_quarter(sb, t // 4)

                sb_loop(0, sb0_fillers())
                for sb in range(1, NSB):
                    sb_loop(sb, sb_fillers(sb))
                # ---- tail ----
                ctx_finish()
                ctxT_issue()
                out_issue(NSB - 1, 0)
                out_issue(NSB - 1, 1)

            if iters == 1:
                _pipeline()
            else:
                with tc.For_i(0, iters):
                    _pipeline()

    _split_sync_waits(nc)
    return nc


def kernel(x, W_Q, b_Q, W_K, b_K, W_V, b_V, W_O, b_O):
    from concourse.bass_utils import run_bass_kernel_spmd

    x = np.asarray(x, np.float32)
    W_Q, b_Q = np.asarray(W_Q, np.float32), np.asarray(b_Q, np.float32)
    W_K, b_K = np.asarray(W_K, np.float32), np.asarray(b_K, np.float32)
    W_V, b_V = np.asarray(W_V, np.float32), np.asarray(b_V, np.float32)
    W_O, b_O = np.asarray(W_O, np.float32), np.asarray(b_O, np.float32)

    if "nc" not in _CACHED:
        _CACHED["nc"] = _build_nc()
    nc = _CACHED["nc"]

    in_maps = []
    for c in range(8):
        b, g = divmod(c, 4)
        in_maps.append(
            {
                "x": np.ascontiguousarray(x[b]),
                "wq": np.ascontiguousarray(W_Q[:, g * GQ : (g + 1) * GQ]),
                "wk": np.ascontiguousarray(W_K[g]),
                "wv": np.ascontiguousarray(W_V[g]),
                "wo": np.ascontiguousarray(W_O[g * GQ : (g + 1) * GQ, :]),
                "bq": np.ascontiguousarray(b_Q[g * GQ : (g + 1) * GQ]),
                "bk": np.ascontiguousarray(b_K[g]),
            }
        )
    res = run_bass_kernel_spmd(nc, in_maps, list(range(8)))

    out = np.zeros((B, S, DM), np.float32)
    for c in range(8):
        b, g = divmod(c, 4)
        out[b] += res.results[c]["out"]
    # host-side bias terms: b_O, plus b_V's exact contribution
    # (softmax rows sum to 1 -> ctx bias = tile(b_V[g]) per head)
    bv_full = np.concatenate([np.tile(b_V[g], H) for g in range(G)])  # [1024]
    out += (b_O + bv_full @ W_O)[None, None, :]
    return out


# revision 5
# speedup vs baseline: 1.4582x; 1.0778x over previous
"""Grouped-Query Attention on 8 Trainium2 NeuronCores.

Sharding: core c handles (batch b = c//4, query-head group g = c%4).
Each core computes its group's Q projection (256 cols of W_Q), the
group-shared K/V projections, 4 heads of attention over the full
sequence, and a partial output projection against the group's 256 rows
of W_O. The host sums the 4 group partials per batch (the "all-reduce")
and adds b_O.

On-core dataflow (all matmuls bf16 operands, fp32 PSUM accumulate):
  xT   = transpose(x)                  PE transpose, fp32 -> bf16 on evac
  QT   = W_Q^T x  (q-dim on partitions), + b_Q on evac
  KT   = W_K^T x  (d_k on partitions), + b_K on evac
  V    = x W_V    (natural [t, d_k])
  S^T  = KT_h^T @ QT_h   per head, [t, s] layout
  P^T  = exp(S^T / 8)    ScalarE, PSUM -> SBUF bf16
  ctx  = P^T_chunk^T @ [V | 1]   natural [s, d_k+1]; col 64 = softmax denom
  ctx /= denom; transpose -> ctxT; out = ctxT^T @ W_O (partial, fp32 out)

b_V and b_O are applied on the host: b_V adds exactly
(tile(b_V) @ W_O_g) to every output row (softmax weights sum to 1).
"""

import numpy as np

S = 2048
DM = 1024
G = 4
H = 4  # heads per group
DK = 64
GQ = 256  # query width per group
B = 2
NK = DM // 128  # 8 contraction chunks
NT = S // 128  # 16 token chunks
SBLK = 512
NSB = S // SBLK  # 4 query super-blocks

_CACHED = {}


def _split_sync_waits(nc, drain_max=1, other_max=1):
    """This walrus build has a single sync-wait slot on CTRL-class
    instructions (Drain/NoOp); Tile's exit drain collects 3+. Move the
    excess onto preceding single-wait NOPs on the same engine."""
    import concourse.mybir as mybir
    import bass_rust

    n_split = 0
    for f in nc.m.functions:
        for bb in f.blocks:
            out = []
            changed = False
            for inst in bb.instructions:
                si = getattr(inst, "sync_info", None)
                limit = drain_max if type(inst).__name__ in ("InstDrain", "InstNoOp") else other_max
                if si is not None and len(si.on_wait) > limit:
                    waits = list(si.on_wait)
                    keep = waits[-limit:] if limit else []
                    head = waits[: len(waits) - limit]
                    for w in head:
                        out.append(
                            mybir.InstNoOp(
                                name=f"{inst.name}-wsp{n_split}",
                                engine=inst.engine,
                                sync_info=mybir.SyncInfo(on_wait=[w], on_update=[]),
                                bass_nofuse=True,
                            )
                        )
                        n_split += 1
                    inst.sync_info = bass_rust.SyncInfo(on_wait=keep, on_update=si.on_update)
                    changed = True
                out.append(inst)
            if changed:
                bb.instructions = out
    return n_split


def _build_nc(iters=1, fp32_tr=False):
    import concourse.bass as bass
    import concourse.mybir as mybir
    import concourse.tile as tile
    from concourse import masks

    F32 = mybir.dt.float32
    BF = mybir.dt.bfloat16

    nc = bass.Bass("TRN2", target_bir_lowering=False, debug=False, num_devices=8)
    x = nc.dram_tensor("x", [S, DM], F32, kind="ExternalInput")
    wq = nc.dram_tensor("wq", [DM, GQ], F32, kind="ExternalInput")
    wk = nc.dram_tensor("wk", [DM, DK], F32, kind="ExternalInput")
    wv = nc.dram_tensor("wv", [DM, DK], F32, kind="ExternalInput")
    wo = nc.dram_tensor("wo", [GQ, DM], F32, kind="ExternalInput")
    bq = nc.dram_tensor("bq", [GQ], F32, kind="ExternalInput")
    bk = nc.dram_tensor("bk", [DK], F32, kind="ExternalInput")
    out = nc.dram_tensor("out", [S, DM], F32, kind="ExternalOutput")

    with tile.TileContext(nc) as tc:
        with (
            tc.tile_pool(name="const", bufs=1) as cpool,
            tc.tile_pool(name="wstg", bufs=3) as wstg,
            tc.tile_pool(name="wts", bufs=1) as wts,
            tc.tile_pool(name="xin", bufs=2) as xin,
            tc.tile_pool(name="acts", bufs=1) as acts,
            tc.tile_pool(name="outp", bufs=2) as outp,
            tc.tile_pool(name="ps_sc", bufs=2, space="PSUM") as ps_sc,
            tc.tile_pool(name="ps_proj", bufs=2, space="PSUM") as ps_proj,
            tc.tile_pool(name="ps_ctx", bufs=2, space="PSUM") as ps_ctx,
        ):
            def _pipeline():
                # ---- constants ----
                ident_f = cpool.tile([128, 128], F32)
                masks.make_identity(nc, ident_f[:])
                ident_b = cpool.tile([128, 128], BF)
                masks.make_identity(nc, ident_b[:])
                bq_t = cpool.tile([128, 2], F32)
                for m in range(2):
                    nc.sync.dma_start(bq_t[:, m : m + 1], bq[m * 128 : (m + 1) * 128])
                bk_t = cpool.tile([64, 1], F32)
                nc.sync.dma_start(bk_t[:], bk[:])

                # ---- weights: stage fp32 (batched 3D-AP DMAs), cast to bf16 ----
                wq_bf = wts.tile([128, NK * GQ], BF)  # chunk k at cols [k*GQ, (k+1)*GQ)
                kv_bf = wts.tile([128, NK * DK * 2], BF)  # wk at k*64, wv at 512+k*64
                wo_bf = wts.tile([128, 2 * DM], BF)  # chunk cj at cols [cj*DM, ...)

                stg = wstg.tile([128, NK * GQ], F32, tag="stg")
                nc.sync.dma_start(
                    stg[:].rearrange("p (k q) -> p k q", q=GQ),
                    wq[:].rearrange("(k p) q -> p k q", p=128),
                )
                nc.vector.tensor_copy(wq_bf[:], stg[:])

                stg2 = wstg.tile([128, NK * GQ], F32, tag="stg")
                nc.sync.dma_start(
                    stg2[:, :512].rearrange("p (k q) -> p k q", q=DK),
                    wk[:].rearrange("(k p) q -> p k q", p=128),
                )
                nc.sync.dma_start(
                    stg2[:, 512:1024].rearrange("p (k q) -> p k q", q=DK),
                    wv[:].rearrange("(k p) q -> p k q", p=128),
                )
                nc.vector.tensor_copy(kv_bf[:], stg2[:, : NK * DK * 2])

                stg3 = wstg.tile([128, NK * GQ], F32, tag="stg")
                nc.sync.dma_start(
                    stg3[:].rearrange("p (c n) -> p c n", n=DM),
                    wo[:].rearrange("(c p) n -> p c n", p=128),
                )
                nc.vector.tensor_copy(wo_bf[:], stg3[:])

                # ---- x load (batched) + transpose (xT[:, k*S + s], bf16) ----
                xT = acts.tile([128, NK * S], BF)
                for sg in range(NSB):
                    xf = xin.tile([128, 4 * DM], F32, tag="xf")
                    nc.sync.dma_start(
                        xf[:].rearrange("p (c d) -> p c d", d=DM),
                        x[sg * SBLK : (sg + 1) * SBLK, :].rearrange("(c p) d -> p c d", p=128),
                    )
                    if fp32_tr:
                        for k in range(NK):
                            ps = ps_proj.tile([128, SBLK], F32, tag="p")
                            for i in range(4):
                                nc.tensor.transpose(
                                    ps[:, i * 128 : (i + 1) * 128],
                                    xf[:, i * DM + k * 128 : i * DM + (k + 1) * 128],
                                    ident_f[:],
                                )
                            nc.vector.tensor_copy(
                                xT[:, k * S + sg * SBLK : k * S + (sg + 1) * SBLK], ps[:]
                            )
                    else:
                        xbf = xin.tile([128, 4 * DM], BF, tag="xbf", bufs=1)
                        nc.vector.tensor_copy(xbf[:], xf[:])
                        for k in range(NK):
                            ps = ps_proj.tile([128, 2 * SBLK], BF, tag="p", name="psx")
                            for i in range(4):
                                nc.tensor.transpose(
                                    ps[:, i * 128 : (i + 1) * 128],
                                    xbf[:, i * DM + k * 128 : i * DM + (k + 1) * 128],
                                    ident_b[:],
                                )
                            nc.vector.tensor_copy(
                                xT[:, k * S + sg * SBLK : k * S + (sg + 1) * SBLK], ps[:, :SBLK]
                            )

                # ---- projections ----
                QT = acts.tile([128, 2 * S], BF)  # m-tile m at cols [m*S, ...): heads 2m, 2m+1
                KT = acts.tile([128, S], BF)  # rows 64-127 duplicate 0-63 (row-tiled scores)
                Vb = acts.tile([128, NT * (DK + 1)], BF)  # [V | ones] per token chunk

                for sg in range(NSB):
                    ps = ps_proj.tile([128, SBLK], F32, tag="p")
                    for k in range(NK):
                        nc.tensor.matmul(
                            ps[:64, :],
                            kv_bf[:, k * DK : (k + 1) * DK],
                            xT[:, k * S + sg * SBLK : k * S + (sg + 1) * SBLK],
                            start=(k == 0),
                            stop=(k == NK - 1),
                        )
                    nc.vector.tensor_scalar_add(
                        KT[:64, sg * SBLK : (sg + 1) * SBLK], ps[:64, :], bk_t[:]
                    )
                nc.sync.dma_start(KT[64:128, :], KT[:64, :])

                for t in range(NT):
                    ps = ps_proj.tile([128, SBLK], F32, tag="p")
                    for k in range(NK):
                        nc.tensor.matmul(
                            ps[:, :DK],
                            xT[:, k * S + t * 128 : k * S + (t + 1) * 128],
                            kv_bf[:, 512 + k * DK : 512 + (k + 1) * DK],
                            start=(k == 0),
                            stop=(k == NK - 1),
                        )
                    nc.vector.tensor_copy(Vb[:, t * 65 : t * 65 + DK], ps[:, :DK])
                nc.vector.memset(Vb[:].rearrange("p (t c) -> p t c", c=65)[:, :, DK], 1.0)

                for m in range(2):
                    for sg in range(NSB):
                        ps = ps_proj.tile([128, SBLK], F32, tag="p")
                        for k in range(NK):
                            nc.tensor.matmul(
                                ps[:],
                                wq_bf[:, k * GQ + m * 128 : k * GQ + (m + 1) * 128],
                                xT[:, k * S + sg * SBLK : k * S + (sg + 1) * SBLK],
                                start=(k == 0),
                                stop=(k == NK - 1),
                            )
                        nc.vector.tensor_scalar_add(
                            QT[:, m * S + sg * SBLK : m * S + (sg + 1) * SBLK], ps[:], bq_t[:, m : m + 1]
                        )

                # ---- attention + output, per query super-block ----
                PT = acts.tile([128, NT * H * SBLK], BF)  # col = (t*H + h)*SBLK + s_local
                ctx_sb = acts.tile([128, 4 * GQ], BF)  # col = sc*GQ + h*DK + d
                ctxT_sb = acts.tile([128, 2 * SBLK], BF)  # col = cj*SBLK + sc*128 + s

                for sb in range(NSB):
                    # scores^T + exp, per (token chunk, head pair); the two heads
                    # of a pair sit at SBUF partitions 0-63 / 64-127 and map to
                    # PE row-tiles (0,0) / (64,0), so their matmuls can overlap.
                    for t in range(NT):
                        for p in range(2):
                            sc = ps_sc.tile([128, 2 * SBLK], F32, tag="sc")
                            for hl in range(2):
                                h = 2 * p + hl
                                nc.tensor.matmul(
                                    sc[:, hl * SBLK : (hl + 1) * SBLK],
                                    KT[hl * 64 : (hl + 1) * 64, t * 128 : (t + 1) * 128],
                                    QT[hl * 64 : (hl + 1) * 64,
                                       p * S + sb * SBLK : p * S + (sb + 1) * SBLK],
                                )
                            nc.scalar.activation(
                                PT[:, (t * H + 2 * p) * SBLK : (t * H + 2 * p + 2) * SBLK],
                                sc[:],
                                mybir.ActivationFunctionType.Exp,
                                scale=0.125,
                            )

                    # ctx natural, one head at a time; col 64 of each group = denom
                    for h in range(H):
                        cps = ps_ctx.tile([128, 4 * (DK + 1)], F32, tag="c")
                        for sc_i in range(4):
                            for t in range(NT):
                                nc.tensor.matmul(
                                    cps[:, sc_i * 65 : sc_i * 65 + 65],
                                    PT[:, (t * H + h) * SBLK + sc_i * 128 : (t * H + h) * SBLK + (sc_i + 1) * 128],
                                    Vb[:, t * 65 : (t + 1) * 65],
                                    start=(t == 0),
                                    stop=(t == NT - 1),
                                )
                        rc = xin.tile([128, 4], F32, tag="rc")
                        nc.vector.reciprocal(
                            rc[:], cps[:].rearrange("p (sc c) -> p sc c", c=65)[:, :, DK]
                        )
                        for sc_i in range(4):
                            nc.vector.tensor_scalar_mul(
                                ctx_sb[:, sc_i * GQ + h * DK : sc_i * GQ + (h + 1) * DK],
                                cps[:, sc_i * 65 : sc_i * 65 + DK],
                                rc[:, sc_i : sc_i + 1],
                            )

                    # transpose ctx -> ctxT
                    for cj in range(2):
                        ps = ps_proj.tile([128, 2 * SBLK], BF, tag="p", name="pst")
                        for sc_i in range(4):
                            nc.tensor.transpose(
                                ps[:, sc_i * 128 : (sc_i + 1) * 128],
                                ctx_sb[:, sc_i * GQ + cj * 128 : sc_i * GQ + (cj + 1) * 128],
                                ident_b[:],
                            )
                        nc.vector.tensor_copy(ctxT_sb[:, cj * SBLK : (cj + 1) * SBLK], ps[:, :SBLK])

                    # output projection (partial over this group's 256 dims)
                    for half in range(2):
                        ot = outp.tile([128, 2 * DM], F32, tag="ot")
                        for ci in range(2):
                            sc_i = half * 2 + ci
                            for nb in range(2):
                                ps = ps_proj.tile([128, SBLK], F32, tag="p")
                                for cj in range(2):
                                    nc.tensor.matmul(
                                        ps[:],
                                        ctxT_sb[:, cj * SBLK + sc_i * 128 : cj * SBLK + (sc_i + 1) * 128],
                                        wo_bf[:, cj * DM + nb * SBLK : cj * DM + (nb + 1) * SBLK],
                                        start=(cj == 0),
                                        stop=(cj == 1),
                                    )
                                nc.vector.tensor_copy(
                                    ot[:, ci * DM + nb * SBLK : ci * DM + (nb + 1) * SBLK], ps[:]
                                )
                        row = sb * SBLK + half * 256
                        nc.sync.dma_start(
                            out[row : row + 256, :].rearrange("(c p) d -> p c d", p=128),
                            ot[:].rearrange("p (c d) -> p c d", d=DM),
                        )

            if iters == 1:
                _pipeline()
            else:
                with tc.For_i(0, iters):
                    _pipeline()

    _split_sync_waits(nc)
    return nc


def kernel(x, W_Q, b_Q, W_K, b_K, W_V, b_V, W_O, b_O):
    from concourse.bass_utils import run_bass_kernel_spmd

    x = np.asarray(x, np.float32)
    W_Q, b_Q = np.asarray(W_Q, np.float32), np.asarray(b_Q, np.float32)
    W_K, b_K = np.asarray(W_K, np.float32), np.asarray(b_K, np.float32)
    W_V, b_V = np.asarray(W_V, np.float32), np.asarray(b_V, np.float32)
    W_O, b_O = np.asarray(W_O, np.float32), np.asarray(b_O, np.float32)

    if "nc" not in _CACHED:
        _CACHED["nc"] = _build_nc()
    nc = _CACHED["nc"]

    in_maps = []
    for c in range(8):
        b, g = divmod(c, 4)
        in_maps.append(
            {
                "x": np.ascontiguousarray(x[b]),
                "wq": np.ascontiguousarray(W_Q[:, g * GQ : (g + 1) * GQ]),
                "wk": np.ascontiguousarray(W_K[g]),
                "wv": np.ascontiguousarray(W_V[g]),
                "wo": np.ascontiguousarray(W_O[g * GQ : (g + 1) * GQ, :]),
                "bq": np.ascontiguousarray(b_Q[g * GQ : (g + 1) * GQ]),
                "bk": np.ascontiguousarray(b_K[g]),
            }
        )
    res = run_bass_kernel_spmd(nc, in_maps, list(range(8)))

    out = np.zeros((B, S, DM), np.float32)
    for c in range(8):
        b, g = divmod(c, 4)
        out[b] += res.results[c]["out"]
    # host-side bias terms: b_O, plus b_V's exact contribution
    # (softmax rows sum to 1 -> ctx bias = tile(b_V[g]) per head)
    bv_full = np.concatenate([np.tile(b_V[g], H) for g in range(G)])  # [1024]
    out += (b_O + bv_full @ W_O)[None, None, :]
    return out

